# revision 35
# baseline (speedup 1.0000x reference)
"""Trainium2 Bass kernel: fused multi-head causal self-attention block.

Computes, for x:(B,S,H), W_qkv:(3H,H), b_qkv:(3H,), W_out:(H,H), b_out:(H,):
    qkv = x @ W_qkv.T + b_qkv ; split into q,k,v heads (NH heads, D=H/NH)
    out = softmax(causal(q k^T / sqrt(D))) v   ; merge heads
    return out @ W_out.T + b_out

Sharding over 8 NeuronCores: DP(2 batches) x TP(4 head-groups).
Core c handles batch b=c//4, head group g=c%4 (heads 4g..4g+3).

v2 design (single fused strip pipeline):
  - All matmul tensors bf16 (same PE rate as fp32r, half the DMA/SBUF,
    FWL-accelerated weight loads). PSUM accumulation stays fp32.
  - x is loaded once, host-prearranged per 512-column strip; Q^T/K^T and
    V projections run as PSUM-chained accumulations (2 banks total).
  - Attention runs strip-outer / head-inner; each strip's normalized A^T
    ([4*128, 512] f16) is AllGather'd across the 4-core batch group as
    soon as the strip finishes, so the output projection for strip s-1
    overlaps attention of strip s (interleaved at head granularity).
  - softmax denominator accumulated on the Vector engine (DVE) and
    contracted with a single ones-vector matmul per (head, strip);
    reciprocal is taken on the [1,512] row then broadcast via a 1-row
    matmul (cheap) instead of a [128,512] reciprocal (very slow).
  - Diagonal score tiles restrict the q-column range to the unmasked
    staircase, skipping fully-masked columns in scores/exp/AV.
Each core computes a disjoint 512-column slice of the output, so the
host does a pure concatenation.
"""

import math

import numpy as np
import ml_dtypes

import concourse.bass as bass
import concourse.mybir as mybir
import concourse.tile as tile
from concourse import bacc, bass_isa
from concourse.bass_utils import run_bass_kernel_spmd

FP = mybir.dt.float32
FR = mybir.dt.float32r
BF = mybir.dt.bfloat16
F16 = mybir.dt.float16

# Full-size problem constants.
B, S, H, NH = 2, 2048, 2048, 16
D = 128
NCORES = 8
GROUPS = 4                  # head-groups per batch (TP degree)
NL = NH // GROUPS           # local heads per core
DG = NL * D                 # per-core slice of the head dim
REPLICA_GROUPS = [[0, 1, 2, 3], [4, 5, 6, 7]]

TRACE = False               # set by test harness to capture NTFF profile
LAST_EXEC_NS = None
LAST_RESULTS = None


def build_nc(s=S, h=H, nh=NH, reps=1, ag=True):
    """Build the SPMD Bass program (identical on all 8 cores)."""
    nc = bacc.Bacc(
        "TRN2",
        target_bir_lowering=False,
        debug=False,
        enable_asserts=False,
        num_devices=NCORES,
    )

    nl = nh // GROUPS
    dg = nl * D
    hc = h // 128               # contraction chunks
    sq = s // 512               # 512-wide strips

    # ---- I/O (all host-prearranged for contiguous DMA) ----------------
    # x strips: [strip, 128, hc, 512] bf16 : x[st, p, c, t] = xT[128c+p, 512st+t]
    x_d = nc.dram_tensor("x", [sq, 128, hc, 512], BF, kind="ExternalInput")
    # weights: [128, hc, dg] : w[p, c, d] = W^T[128c+p, d]
    wq_d = nc.dram_tensor("wq", [128, hc, dg], BF, kind="ExternalInput")
    wk_d = nc.dram_tensor("wk", [128, hc, dg], BF, kind="ExternalInput")
    wv_d = nc.dram_tensor("wv", [128, hc, dg], BF, kind="ExternalInput")
    # out-proj weights, rows permuted to AG order: chunk c=(4r+l) <-> head 4r+l
    wo_d = nc.dram_tensor("wo", [128, hc, dg], F16, kind="ExternalInput")
    bq_d = nc.dram_tensor("bq", [128, nl], FP, kind="ExternalInput")
    bk_d = nc.dram_tensor("bk", [128, nl], FP, kind="ExternalInput")
    bv_d = nc.dram_tensor("bv", [128, dg], FP, kind="ExternalInput")
    bo_d = nc.dram_tensor("bo", [128, dg], FP, kind="ExternalInput")
    mask_d = nc.dram_tensor("mask", [128, 896], BF, kind="ExternalInput")
    ones_d = nc.dram_tensor("ones", [128, 128], FR, kind="ExternalInput")
    out_d = nc.dram_tensor("out", [s, dg], FP, kind="ExternalOutput")

    with tile.TileContext(nc) as tc:
        with tc.tile_pool(name="const", bufs=1) as constp:
            mask_sb = constp.tile([128, 896], BF)
            nc.sync.dma_start(mask_sb[:], mask_d[:])
            ones_sb = constp.tile([128, 128], FR)
            nc.sync.dma_start(ones_sb[:], ones_d[:])
            bq_sb = constp.tile([128, nl], FP)
            nc.sync.dma_start(bq_sb[:], bq_d[:])
            bk_sb = constp.tile([128, nl], FP)
            nc.sync.dma_start(bk_sb[:], bk_d[:])
            bv_sb = constp.tile([128, dg], FP)
            nc.sync.dma_start(bv_sb[:], bv_d[:])
            bo_sb = constp.tile([128, dg], FP)
            nc.sync.dma_start(bo_sb[:], bo_d[:])

            for _rep in range(reps):
                _emit_body(nc, tc, s, h, nh,
                           x_d, wq_d, wk_d, wv_d, wo_d, out_d,
                           bq_sb, bk_sb, bv_sb, bo_sb,
                           mask_sb, ones_sb, ag)

    nc.compile()
    return nc


def _emit_body(nc, tc, s, h, nh,
               x_d, wq_d, wk_d, wv_d, wo_d, out_d,
               bq_sb, bk_sb, bv_sb, bo_sb, mask_sb, ones_sb, ag=True):
    nl = nh // GROUPS
    dg = nl * D
    hc = h // 128
    sq = s // 512
    st_n = s // 128
    scale = 1.0 / math.sqrt(D)
    ones_col = ones_sb[:, 0:1]   # [128,1] FR: denominator partition-contract
    ones_row = ones_sb[0:1, :]   # [1,128] FR: partition broadcast

    with tc.tile_pool(name="wts", bufs=1) as wtp, \
         tc.tile_pool(name="xp", bufs=2) as xp, \
         tc.tile_pool(name="qkv", bufs=1) as qkvp, \
         tc.tile_pool(name="atrp", bufs=2) as atrp, \
         tc.tile_pool(name="etp", bufs=3) as etp, \
         tc.tile_pool(name="anp", bufs=2) as anp, \
         tc.tile_pool(name="dnp", bufs=2) as dnp, \
         tc.tile_pool(name="obp", bufs=1) as obp, \
         tc.tile_pool(name="dramp", bufs=1, space="DRAM") as dramp, \
         tc.tile_pool(name="psCH", bufs=2, space="PSUM") as psCH, \
         tc.tile_pool(name="psS", bufs=2, space="PSUM") as psS, \
         tc.tile_pool(name="psAV", bufs=2, space="PSUM") as psAV, \
         tc.tile_pool(name="psDR", bufs=1, space="PSUM") as psDR, \
         tc.tile_pool(name="psO", bufs=1, space="PSUM") as psO:

        # ---- persistent SBUF tensors -----------------------------------
        qT = [qkvp.tile([128, s], BF, tag=f"qT{t}", name=f"qT{t}") for t in range(nl)]
        kT = [qkvp.tile([128, s], BF, tag=f"kT{t}", name=f"kT{t}") for t in range(nl)]
        vv = [qkvp.tile([128, dg], BF, tag=f"v{t}", name=f"v{t}") for t in range(st_n)]

        x_sbs = {}

        def emit_x_load(strip):
            x_sb = xp.tile([128, hc, 512], BF, tag="xs", name="xs")
            # per-chunk sub-DMAs so the first chains can start early
            for c in range(hc):
                nc.sync.dma_start(x_sb[:, c, :], x_d[strip, :, c, :])
            x_sbs[strip] = x_sb

        # interleave strip-0 x with the Q/K weights so the first chain's
        # per-chunk matmuls can start as soon as their operands land
        wq_sb = wtp.tile([128, hc, dg], BF, tag="wq", name="wq_sb")
        wk_sb = wtp.tile([128, hc, dg], BF, tag="wk", name="wk_sb")
        wv_sb = wtp.tile([128, hc, dg], BF, tag="wv", name="wv_sb")
        wo_sb = wtp.tile([128, hc, dg], F16, tag="wo", name="wo_sb")
        x0_sb = xp.tile([128, hc, 512], BF, tag="xs", name="xs0")
        for b in range(4):
            cb = slice(4 * b, 4 * b + 4)
            nc.sync.dma_start(wq_sb[:, cb, :], wq_d[:, cb, :])
            for c in range(4 * b, 4 * b + 4):
                nc.sync.dma_start(x0_sb[:, c, :], x_d[0, :, c, :])
        x_sbs[0] = x0_sb
        nc.sync.dma_start(wk_sb[:], wk_d[:])
        nc.scalar.dma_start(wv_sb[:], wv_d[:])
        nc.scalar.dma_start(wo_sb[:], wo_d[:])

        # AG buffers (DRAM), split by head pair: pair p holds heads 2p,2p+1
        agin = [[dramp.tile([2 * 128, 512], F16, tag=f"agin{st}_{p}",
                            name=f"agin{st}_{p}") for p in range(2)]
                for st in range(sq)]
        agout = [[dramp.tile([4 * 256, 512], F16, tag=f"agout{st}_{p}",
                             name=f"agout{st}_{p}") for p in range(2)]
                 for st in range(sq)]

        def emit_qkv(strip):
            """Q^T,K^T,V projections for one 512-col strip of the sequence."""
            cs = slice(512 * strip, 512 * strip + 512)
            x_sb = x_sbs.pop(strip)
            # Q/K chains: one [128,512] psum accumulated over all hc chunks
            for gi in range(2 * nl):
                is_q = gi % 2 == 0          # interleave Q,K per head
                t = gi // 2
                w_sb = wq_sb if is_q else wk_sb
                ps = psCH.tile([128, 512], FP, tag="chain", name="ps_qk")
                for c in range(hc):
                    nc.tensor.matmul(
                        ps[:],
                        w_sb[:, c, 128 * t:128 * t + 128],
                        x_sb[:, c, :],
                        start=(c == 0), stop=(c == hc - 1),
                    )
                if is_q:
                    nc.scalar.activation(
                        qT[t][:, cs], ps[:],
                        mybir.ActivationFunctionType.Identity,
                        bias=bq_sb[:, t:t + 1],
                    )
                else:
                    nc.vector.tensor_scalar_add(kT[t][:, cs], ps[:], bk_sb[:, t:t + 1])
            # V chains: natural [s,d] layout, one per 128-row s-tile
            for sti in range(4):
                st_idx = 4 * strip + sti
                ps = psCH.tile([128, dg], FP, tag="chain", name="ps_v")
                for c in range(hc):
                    nc.tensor.matmul(
                        ps[:],
                        x_sb[:, c, 128 * sti:128 * sti + 128],
                        wv_sb[:, c, :],
                        start=(c == 0), stop=(c == hc - 1),
                    )
                nc.vector.tensor_add(vv[st_idx][:], ps[:], bv_sb[:])

        def emit_attention_head(strip, l):
            """Causal attention for head l restricted to q-strip `strip`."""
            nk = 4 * strip + 4
            ps_av = psAV.tile([128, 512], FP, tag="ps_av", name="ps_av")
            dn_acc = dnp.tile([128, 512], FR, tag="dn_acc", name="dn_acc")
            ets = []
            # software-pipelined: scores(kt) ... AV(kt-1) so exp can run ahead
            for kt in range(nk + 1):
                if kt < nk:
                    c = kt - 4 * strip          # >=0 on diagonal tiles
                    qc = slice(128 * c, 512) if c >= 0 else slice(0, 512)
                    qg = slice(512 * strip + qc.start, 512 * strip + 512)
                    ps_s = psS.tile([128, 512], FP, tag="ps_s", name="ps_s")
                    nc.tensor.matmul(
                        ps_s[:, qc],
                        kT[l][:, 128 * kt:128 * kt + 128],
                        qT[l][:, qg],
                        start=True, stop=True,
                    )
                    et = etp.tile([128, 512], BF, tag="et", name="et")
                    nc.scalar.activation(
                        et[:, qc], ps_s[:, qc],
                        mybir.ActivationFunctionType.Exp,
                        scale=scale,
                    )
                    if c >= 0:
                        nc.vector.tensor_mul(
                            et[:, qc], et[:, qc], mask_sb[:, 384:896 - 128 * c])
                    # denominator accumulate on DVE
                    if kt == 0:
                        nc.vector.tensor_copy(dn_acc[:, qc], et[:, qc])
                    else:
                        nc.vector.tensor_add(dn_acc[:, qc], dn_acc[:, qc], et[:, qc])
                    ets.append((et, qc))
                if kt >= 1:
                    et, qc = ets[kt - 1]
                    nc.tensor.matmul(
                        ps_av[:, qc],
                        vv[kt - 1][:, 128 * l:128 * l + 128],
                        et[:, qc],
                        start=(kt - 1 == 0), stop=(kt - 1 == nk - 1),
                    )
            # denominator: ones-contract over partitions (PE), fast approx
            # reciprocal on the [1,512] row (DVE), ones-broadcast back to
            # [128,512] (PE), then normalize. ACT stages the PSUM->SBUF hops.
            ps_dn = psDR.tile([1, 512], FP, tag="dnrb", name="ps_dn")
            nc.tensor.matmul(ps_dn[:], ones_col, dn_acc[:], start=True, stop=True)
            dn_f32 = dnp.tile([1, 512], FP, tag="dn_f32", name="dn_f32")
            nc.scalar.copy(dn_f32[:], ps_dn[:])
            rec_f32 = dnp.tile([1, 512], FP, tag="rec_f32", name="rec_f32")
            nc.vector.reciprocal_approx_fast(rec_f32[:], dn_f32[:])
            rec_fr = dnp.tile([1, 512], FR, tag="rec_fr", name="rec_fr")
            nc.vector.tensor_copy(rec_fr[:], rec_f32[:])
            ps_rb = psDR.tile([128, 512], FP, tag="dnrb", name="ps_rb")
            nc.tensor.matmul(ps_rb[:], ones_row, rec_fr[:], start=True, stop=True)
            rb_sb = dnp.tile([128, 512], FP, tag="rb_sb", name="rb_sb", bufs=1)
            nc.scalar.copy(rb_sb[:], ps_rb[:])
            an = anp.tile([128, 512], F16, tag="an", name="an")
            nc.vector.tensor_mul(an[:], ps_av[:], rb_sb[:])
            # ship this head's strip slice to the AG input buffer
            nc.sync.dma_start(
                agin[strip][l // 2][128 * (l % 2):128 * (l % 2) + 128, :], an[:])

        at_p = {}

        def emit_ag(strip, pair):
            """Trigger the AllGather for one head pair, then immediately queue
            the SBUF loads of its 4 rank blocks. Both live on the gpsimd
            queue: nothing else depends on collective completion, so the
            completion wait blocks nothing."""
            if ag:
                nc.gpsimd.collective_compute(
                    "AllGather",
                    mybir.AluOpType.bypass,
                    replica_groups=REPLICA_GROUPS,
                    ins=[agin[strip][pair].opt()],
                    outs=[agout[strip][pair].opt()],
                )
            else:
                nc.gpsimd.dma_start(agout[strip][pair][0:256, :],
                                    agin[strip][pair][:])
            for r in range(GROUPS):
                t = atrp.tile([128, 2, 512], F16, tag=f"atr{pair}{r}",
                              name=f"atr{pair}{r}")
                nc.gpsimd.dma_start(
                    t[:],
                    agout[strip][pair][256 * r:256 * r + 256, :]
                    .rearrange("(l p) t -> p l t", p=128))
                at_p[(strip, pair, r)] = t

        def emit_outproj_sti(strip, sti):
            """One 128-row s-tile of the output projection for `strip`."""
            rs = slice(512 * strip + 128 * sti, 512 * strip + 128 * sti + 128)
            ssl = slice(128 * sti, 128 * sti + 128)
            ps_o = psO.tile([128, dg], FP, tag="ps_o", name="ps_o")
            for c in range(hc):
                pair, rl = divmod(c, 2 * GROUPS)
                r, lp = divmod(rl, 2)
                lhsT = at_p[(strip, pair, r)][:, lp, ssl]
                nc.tensor.matmul(
                    ps_o[:], lhsT, wo_sb[:, c, :],
                    start=(c == 0), stop=(c == hc - 1),
                )
            ob = obp.tile([128, dg], FP, tag="ob", name="ob")
            nc.vector.tensor_add(ob[:], ps_o[:], bo_sb[:])
            nc.sync.dma_start(out_d[rs, :], ob[:])

        # ---- main fused pipeline ---------------------------------------
        # Each head pair's AllGather fires as soon as both heads finish, and
        # outproj for strip s runs right after strip s+1's attention — after
        # its atr loads are emitted (and long since landed), before strip
        # s+1's own atr loads, so queue-threshold waits stay tight.
        for strip in range(sq):
            if strip + 1 < sq:
                emit_x_load(strip + 1)
            emit_qkv(strip)
            if strip >= 1:
                for sti in range(4):
                    emit_outproj_sti(strip - 1, sti)
            for l in range(nl):
                emit_attention_head(strip, l)
                if l == 1:
                    emit_ag(strip, 0)
            emit_ag(strip, 1)
        for sti in range(4):
            emit_outproj_sti(sq - 1, sti)


def make_inputs(x, W_qkv, b_qkv, W_out, b_out, s=S, h=H, nh=NH):
    """Host-side sharding: per-core input dicts (layout prep only)."""
    nl = nh // GROUPS
    dg = nl * D
    hc = h // 128
    sq = s // 512
    bf16 = ml_dtypes.bfloat16
    x = np.asarray(x, dtype=np.float32)
    W_qkv = np.asarray(W_qkv, dtype=np.float32)
    b_qkv = np.asarray(b_qkv, dtype=np.float32)
    W_out = np.asarray(W_out, dtype=np.float32)
    b_out = np.asarray(b_out, dtype=np.float32)

    # causal staircase master mask: mask[i, u] = 1 iff u >= i + 384
    uu = np.arange(896)[None, :]
    ii = np.arange(128)[:, None]
    mask = (uu >= ii + 384).astype(bf16)
    ones = np.ones((128, 128), dtype=np.float32)

    WoT = W_out.T  # [h (d-in), h (n-out)]
    in_maps = []
    for core in range(NCORES):
        b, g = divmod(core, GROUPS)
        xT = x[b].T                                   # [h, s]
        # x strips: [sq, 128, hc, 512]
        xs = np.ascontiguousarray(
            xT.reshape(hc, 128, sq, 512).transpose(2, 1, 0, 3).astype(bf16))

        def arr_w(wslice, dt):
            # [dg, h] -> transposed chunks [128, hc, dg]
            return np.ascontiguousarray(
                wslice.T.reshape(hc, 128, dg).transpose(1, 0, 2).astype(dt))

        wq = arr_w(W_qkv[dg * g:dg * (g + 1), :], bf16)
        wk = arr_w(W_qkv[h + dg * g:h + dg * (g + 1), :], bf16)
        wv = arr_w(W_qkv[2 * h + dg * g:2 * h + dg * (g + 1), :], bf16)
        bq = np.ascontiguousarray(
            b_qkv[dg * g:dg * (g + 1)].reshape(nl, 128).T)      # [128, nl]
        bk = np.ascontiguousarray(
            b_qkv[h + dg * g:h + dg * (g + 1)].reshape(nl, 128).T)
        bv = np.tile(b_qkv[2 * h + dg * g:2 * h + dg * (g + 1)][None, :], (128, 1))
        bo = np.tile(b_out[dg * g:dg * (g + 1)][None, :], (128, 1))
        # W_out^T rows permuted to the per-pair AG d-order:
        # chunk c = 8*pair + 2*r + lp  <->  global head 4r + 2*pair + lp
        blocks = []
        for pair in range(2):
            for r in range(GROUPS):
                for lp in range(2):
                    hh = nl * r + 2 * pair + lp
                    blocks.append(WoT[D * hh:D * (hh + 1), dg * g:dg * (g + 1)])
        wo = np.ascontiguousarray(
            np.concatenate(blocks, axis=0)
            .reshape(hc, 128, dg).transpose(1, 0, 2).astype(np.float16))
        in_maps.append({
            "x": xs, "wq": wq, "wk": wk, "wv": wv, "wo": wo,
            "bq": bq, "bk": bk,
            "bv": np.ascontiguousarray(bv.astype(np.float32)),
            "bo": np.ascontiguousarray(bo.astype(np.float32)),
            "mask": mask, "ones": ones,
        })
    return in_maps


_NC_CACHE = {}


def _get_nc(key=(S, H, NH)):
    if key not in _NC_CACHE:
        _NC_CACHE[key] = build_nc(*key)
    return _NC_CACHE[key]


def kernel(x, W_qkv, b_qkv, W_out, b_out):
    global LAST_EXEC_NS, LAST_RESULTS
    nc = _get_nc()
    in_maps = make_inputs(x, W_qkv, b_qkv, W_out, b_out)
    res = run_bass_kernel_spmd(
        nc, in_maps, core_ids=list(range(NCORES)), trace=TRACE)
    LAST_EXEC_NS = res.exec_time_ns
    LAST_RESULTS = res
    out = np.empty((B, S, H), dtype=np.float32)
    for core in range(NCORES):
        b, g = divmod(core, GROUPS)
        out[b, :, DG * g:DG * (g + 1)] = res.results[core]["out"]
    return out


# revision 36
# speedup vs baseline: 1.0770x; 1.0770x over previous
"""Trainium2 Bass kernel: fused multi-head causal self-attention block.

Computes, for x:(B,S,H), W_qkv:(3H,H), b_qkv:(3H,), W_out:(H,H), b_out:(H,):
    qkv = x @ W_qkv.T + b_qkv ; split into q,k,v heads (NH heads, D=H/NH)
    out = softmax(causal(q k^T / sqrt(D))) v   ; merge heads
    return out @ W_out.T + b_out

Sharding over 8 NeuronCores: DP(2 batches) x TP(4 head-groups).
Core c handles batch b=c//4, head group g=c%4 (heads 4g..4g+3).

v2 design (single fused strip pipeline):
  - All matmul tensors bf16 (same PE rate as fp32r, half the DMA/SBUF,
    FWL-accelerated weight loads). PSUM accumulation stays fp32.
  - x is loaded once, host-prearranged per 512-column strip; Q^T/K^T and
    V projections run as PSUM-chained accumulations (2 banks total).
  - Attention runs strip-outer / head-inner; each strip's normalized A^T
    ([4*128, 512] f16) is AllGather'd across the 4-core batch group as
    soon as the strip finishes, so the output projection for strip s-1
    overlaps attention of strip s (interleaved at head granularity).
  - softmax denominator accumulated on the Vector engine (DVE) and
    contracted with a single ones-vector matmul per (head, strip);
    reciprocal is taken on the [1,512] row then broadcast via a 1-row
    matmul (cheap) instead of a [128,512] reciprocal (very slow).
  - Diagonal score tiles restrict the q-column range to the unmasked
    staircase, skipping fully-masked columns in scores/exp/AV.
Each core computes a disjoint 512-column slice of the output, so the
host does a pure concatenation.
"""

import math

import numpy as np
import ml_dtypes

import concourse.bass as bass
import concourse.mybir as mybir
import concourse.tile as tile
from concourse import bacc, bass_isa
from concourse.bass_utils import run_bass_kernel_spmd

FP = mybir.dt.float32
FR = mybir.dt.float32r
BF = mybir.dt.bfloat16
F16 = mybir.dt.float16

# Full-size problem constants.
B, S, H, NH = 2, 2048, 2048, 16
D = 128
NCORES = 8
GROUPS = 4                  # head-groups per batch (TP degree)
NL = NH // GROUPS           # local heads per core
DG = NL * D                 # per-core slice of the head dim
REPLICA_GROUPS = [[0, 1, 2, 3], [4, 5, 6, 7]]

TRACE = False               # set by test harness to capture NTFF profile
LAST_EXEC_NS = None
LAST_RESULTS = None


def build_nc(s=S, h=H, nh=NH, reps=1, ag=True):
    """Build the SPMD Bass program (identical on all 8 cores)."""
    nc = bacc.Bacc(
        "TRN2",
        target_bir_lowering=False,
        debug=False,
        enable_asserts=False,
        num_devices=NCORES,
    )

    nl = nh // GROUPS
    dg = nl * D
    hc = h // 128               # contraction chunks
    sq = s // 512               # 512-wide strips

    # ---- I/O (all host-prearranged for contiguous DMA) ----------------
    # x strips: [strip, 128, hc, 512] bf16 : x[st, p, c, t] = xT[128c+p, 512st+t]
    x_d = nc.dram_tensor("x", [sq, 128, hc, 512], BF, kind="ExternalInput")
    # weights: [128, hc, dg] : w[p, c, d] = W^T[128c+p, d]
    wq_d = nc.dram_tensor("wq", [128, hc, dg], BF, kind="ExternalInput")
    wk_d = nc.dram_tensor("wk", [128, hc, dg], BF, kind="ExternalInput")
    wv_d = nc.dram_tensor("wv", [128, hc, dg], BF, kind="ExternalInput")
    # out-proj weights, rows permuted to AG order: chunk c=(4r+l) <-> head 4r+l
    wo_d = nc.dram_tensor("wo", [128, hc, dg], F16, kind="ExternalInput")
    bq_d = nc.dram_tensor("bq", [128, nl], FP, kind="ExternalInput")
    bk_d = nc.dram_tensor("bk", [128, nl], FP, kind="ExternalInput")
    bv_d = nc.dram_tensor("bv", [128, dg], FP, kind="ExternalInput")
    bo_d = nc.dram_tensor("bo", [128, dg], FP, kind="ExternalInput")
    mask_d = nc.dram_tensor("mask", [128, 896], BF, kind="ExternalInput")
    ones_d = nc.dram_tensor("ones", [128, 128], FR, kind="ExternalInput")
    out_d = nc.dram_tensor("out", [s, dg], FP, kind="ExternalOutput")

    with tile.TileContext(nc) as tc:
        with tc.tile_pool(name="const", bufs=1) as constp:
            mask_sb = constp.tile([128, 896], BF)
            nc.sync.dma_start(mask_sb[:], mask_d[:])
            ones_sb = constp.tile([128, 128], FR)
            nc.sync.dma_start(ones_sb[:], ones_d[:])
            bq_sb = constp.tile([128, nl], FP)
            nc.sync.dma_start(bq_sb[:], bq_d[:])
            bk_sb = constp.tile([128, nl], FP)
            nc.sync.dma_start(bk_sb[:], bk_d[:])
            bv_sb = constp.tile([128, dg], FP)
            nc.sync.dma_start(bv_sb[:], bv_d[:])
            bo_sb = constp.tile([128, dg], FP)
            nc.sync.dma_start(bo_sb[:], bo_d[:])

            for _rep in range(reps):
                _emit_body(nc, tc, s, h, nh,
                           x_d, wq_d, wk_d, wv_d, wo_d, out_d,
                           bq_sb, bk_sb, bv_sb, bo_sb,
                           mask_sb, ones_sb, ag)

    nc.compile()
    return nc


def _emit_body(nc, tc, s, h, nh,
               x_d, wq_d, wk_d, wv_d, wo_d, out_d,
               bq_sb, bk_sb, bv_sb, bo_sb, mask_sb, ones_sb, ag=True):
    nl = nh // GROUPS
    dg = nl * D
    hc = h // 128
    sq = s // 512
    st_n = s // 128
    scale = 1.0 / math.sqrt(D)
    ones_col = ones_sb[:, 0:1]   # [128,1] FR: denominator partition-contract
    ones_row = ones_sb[0:1, :]   # [1,128] FR: partition broadcast

    with tc.tile_pool(name="wts", bufs=1) as wtp, \
         tc.tile_pool(name="xp", bufs=2) as xp, \
         tc.tile_pool(name="qkv", bufs=1) as qkvp, \
         tc.tile_pool(name="atrp", bufs=2) as atrp, \
         tc.tile_pool(name="etp", bufs=3) as etp, \
         tc.tile_pool(name="anp", bufs=2) as anp, \
         tc.tile_pool(name="dnp", bufs=2) as dnp, \
         tc.tile_pool(name="obp", bufs=1) as obp, \
         tc.tile_pool(name="dramp", bufs=1, space="DRAM") as dramp, \
         tc.tile_pool(name="psCH", bufs=2, space="PSUM") as psCH, \
         tc.tile_pool(name="psS", bufs=2, space="PSUM") as psS, \
         tc.tile_pool(name="psAV", bufs=2, space="PSUM") as psAV, \
         tc.tile_pool(name="psDR", bufs=1, space="PSUM") as psDR, \
         tc.tile_pool(name="psO", bufs=1, space="PSUM") as psO:

        # ---- persistent SBUF tensors -----------------------------------
        qT = [qkvp.tile([128, s], BF, tag=f"qT{t}", name=f"qT{t}") for t in range(nl)]
        kT = [qkvp.tile([128, s], BF, tag=f"kT{t}", name=f"kT{t}") for t in range(nl)]
        vv = [qkvp.tile([128, dg], BF, tag=f"v{t}", name=f"v{t}") for t in range(st_n)]

        x_sbs = {}

        def emit_x_load(strip):
            x_sb = xp.tile([128, hc, 512], BF, tag="xs", name="xs")
            # per-chunk sub-DMAs so the first chains can start early
            for c in range(hc):
                nc.sync.dma_start(x_sb[:, c, :], x_d[strip, :, c, :])
            x_sbs[strip] = x_sb

        # interleave strip-0 x with the Q/K weights so the first chain's
        # per-chunk matmuls can start as soon as their operands land
        wq_sb = wtp.tile([128, hc, dg], BF, tag="wq", name="wq_sb")
        wk_sb = wtp.tile([128, hc, dg], BF, tag="wk", name="wk_sb")
        wv_sb = wtp.tile([128, hc, dg], BF, tag="wv", name="wv_sb")
        wo_sb = wtp.tile([128, hc, dg], F16, tag="wo", name="wo_sb")
        x0_sb = xp.tile([128, hc, 512], BF, tag="xs", name="xs0")
        for b in range(4):
            cb = slice(4 * b, 4 * b + 4)
            nc.sync.dma_start(wq_sb[:, cb, :], wq_d[:, cb, :])
            for c in range(4 * b, 4 * b + 4):
                nc.sync.dma_start(x0_sb[:, c, :], x_d[0, :, c, :])
        x_sbs[0] = x0_sb
        nc.sync.dma_start(wk_sb[:], wk_d[:])
        nc.scalar.dma_start(wv_sb[:], wv_d[:])
        nc.scalar.dma_start(wo_sb[:], wo_d[:])

        # AG buffers (DRAM), split by head pair: pair p holds heads 2p,2p+1
        agin = [[dramp.tile([2 * 128, 512], F16, tag=f"agin{st}_{p}",
                            name=f"agin{st}_{p}") for p in range(2)]
                for st in range(sq)]
        agout = [[dramp.tile([4 * 256, 512], F16, tag=f"agout{st}_{p}",
                             name=f"agout{st}_{p}") for p in range(2)]
                 for st in range(sq)]

        def emit_qkv(strip):
            """Q^T,K^T,V projections for one 512-col strip of the sequence."""
            cs = slice(512 * strip, 512 * strip + 512)
            x_sb = x_sbs.pop(strip)
            # Q/K chains: one [128,512] psum accumulated over all hc chunks
            for gi in range(2 * nl):
                is_q = gi % 2 == 0          # interleave Q,K per head
                t = gi // 2
                w_sb = wq_sb if is_q else wk_sb
                ps = psCH.tile([128, 512], FP, tag="chain", name="ps_qk")
                for c in range(hc):
                    nc.tensor.matmul(
                        ps[:],
                        w_sb[:, c, 128 * t:128 * t + 128],
                        x_sb[:, c, :],
                        start=(c == 0), stop=(c == hc - 1),
                    )
                if is_q:
                    nc.scalar.activation(
                        qT[t][:, cs], ps[:],
                        mybir.ActivationFunctionType.Identity,
                        bias=bq_sb[:, t:t + 1],
                    )
                else:
                    nc.vector.tensor_scalar_add(kT[t][:, cs], ps[:], bk_sb[:, t:t + 1])
            # V chains: natural [s,d] layout, one per 128-row s-tile
            for sti in range(4):
                st_idx = 4 * strip + sti
                ps = psCH.tile([128, dg], FP, tag="chain", name="ps_v")
                for c in range(hc):
                    nc.tensor.matmul(
                        ps[:],
                        x_sb[:, c, 128 * sti:128 * sti + 128],
                        wv_sb[:, c, :],
                        start=(c == 0), stop=(c == hc - 1),
                    )
                nc.vector.tensor_add(vv[st_idx][:], ps[:], bv_sb[:])

        def emit_attention_head(strip, l):
            """Causal attention for head l restricted to q-strip `strip`."""
            nk = 4 * strip + 4
            ps_av = psAV.tile([128, 512], FP, tag="ps_av", name="ps_av")
            dn_acc = dnp.tile([128, 512], FR, tag="dn_acc", name="dn_acc")
            ets = []
            # software-pipelined: scores(kt) ... AV(kt-1) so exp can run ahead
            for kt in range(nk + 1):
                if kt < nk:
                    c = kt - 4 * strip          # >=0 on diagonal tiles
                    qc = slice(128 * c, 512) if c >= 0 else slice(0, 512)
                    qg = slice(512 * strip + qc.start, 512 * strip + 512)
                    ps_s = psS.tile([128, 512], FP, tag="ps_s", name="ps_s")
                    nc.tensor.matmul(
                        ps_s[:, qc],
                        kT[l][:, 128 * kt:128 * kt + 128],
                        qT[l][:, qg],
                        start=True, stop=True,
                    )
                    et = etp.tile([128, 512], BF, tag="et", name="et")
                    nc.scalar.activation(
                        et[:, qc], ps_s[:, qc],
                        mybir.ActivationFunctionType.Exp,
                        scale=scale,
                    )
                    if c >= 0:
                        nc.vector.tensor_mul(
                            et[:, qc], et[:, qc], mask_sb[:, 384:896 - 128 * c])
                    # denominator accumulate on DVE
                    if kt == 0:
                        nc.vector.tensor_copy(dn_acc[:, qc], et[:, qc])
                    else:
                        nc.vector.tensor_add(dn_acc[:, qc], dn_acc[:, qc], et[:, qc])
                    ets.append((et, qc))
                if kt >= 1:
                    et, qc = ets[kt - 1]
                    nc.tensor.matmul(
                        ps_av[:, qc],
                        vv[kt - 1][:, 128 * l:128 * l + 128],
                        et[:, qc],
                        start=(kt - 1 == 0), stop=(kt - 1 == nk - 1),
                    )
            # denominator: ones-contract over partitions (PE), fast approx
            # reciprocal on the [1,512] row (DVE), ones-broadcast back to
            # [128,512] (PE), then normalize. ACT stages the PSUM->SBUF hops.
            ps_dn = psDR.tile([1, 512], FP, tag="dnrb", name="ps_dn")
            nc.tensor.matmul(ps_dn[:], ones_col, dn_acc[:], start=True, stop=True)
            dn_f32 = dnp.tile([1, 512], FP, tag="dn_f32", name="dn_f32")
            nc.scalar.copy(dn_f32[:], ps_dn[:])
            rec_f32 = dnp.tile([1, 512], FP, tag="rec_f32", name="rec_f32")
            nc.vector.reciprocal_approx_fast(rec_f32[:], dn_f32[:])
            rec_fr = dnp.tile([1, 512], FR, tag="rec_fr", name="rec_fr")
            nc.vector.tensor_copy(rec_fr[:], rec_f32[:])
            ps_rb = psDR.tile([128, 512], FP, tag="dnrb", name="ps_rb")
            nc.tensor.matmul(ps_rb[:], ones_row, rec_fr[:], start=True, stop=True)
            rb_sb = dnp.tile([128, 512], FP, tag="rb_sb", name="rb_sb", bufs=1)
            nc.scalar.copy(rb_sb[:], ps_rb[:])
            an = anp.tile([128, 512], F16, tag="an", name="an")
            nc.vector.tensor_mul(an[:], ps_av[:], rb_sb[:])
            # ship this head's strip slice to the AG input buffer
            nc.sync.dma_start(
                agin[strip][l // 2][128 * (l % 2):128 * (l % 2) + 128, :], an[:])

        at_p = {}

        def emit_ag(strip, pair):
            """Trigger the AllGather for one head pair, then immediately queue
            the SBUF loads of its 4 rank blocks. Both live on the gpsimd
            queue: nothing else depends on collective completion, so the
            completion wait blocks nothing."""
            if ag:
                nc.gpsimd.collective_compute(
                    "AllGather",
                    mybir.AluOpType.bypass,
                    replica_groups=REPLICA_GROUPS,
                    ins=[agin[strip][pair].opt()],
                    outs=[agout[strip][pair].opt()],
                )
            else:
                nc.gpsimd.dma_start(agout[strip][pair][0:256, :],
                                    agin[strip][pair][:])
            for r in range(GROUPS):
                t = atrp.tile([128, 2, 512], F16, tag=f"atr{pair}{r}",
                              name=f"atr{pair}{r}")
                nc.gpsimd.dma_start(
                    t[:],
                    agout[strip][pair][256 * r:256 * r + 256, :]
                    .rearrange("(l p) t -> p l t", p=128))
                at_p[(strip, pair, r)] = t

        def emit_outproj_sti(strip, sti):
            """One 128-row s-tile of the output projection for `strip`."""
            rs = slice(512 * strip + 128 * sti, 512 * strip + 128 * sti + 128)
            ssl = slice(128 * sti, 128 * sti + 128)
            ps_o = psO.tile([128, dg], FP, tag="ps_o", name="ps_o")
            for c in range(hc):
                pair, rl = divmod(c, 2 * GROUPS)
                r, lp = divmod(rl, 2)
                lhsT = at_p[(strip, pair, r)][:, lp, ssl]
                nc.tensor.matmul(
                    ps_o[:], lhsT, wo_sb[:, c, :],
                    start=(c == 0), stop=(c == hc - 1),
                )
            ob = obp.tile([128, dg], FP, tag="ob", name="ob")
            nc.vector.tensor_add(ob[:], ps_o[:], bo_sb[:])
            nc.sync.dma_start(out_d[rs, :], ob[:])

        # ---- main fused pipeline ---------------------------------------
        # Each head pair's AllGather fires as soon as both heads finish, and
        # outproj for strip s runs right after strip s+1's attention — after
        # its atr loads are emitted (and long since landed), before strip
        # s+1's own atr loads, so queue-threshold waits stay tight.
        for strip in range(sq):
            if strip + 1 < sq:
                emit_x_load(strip + 1)
            emit_qkv(strip)
            if strip >= 2:
                for sti in range(4):
                    emit_outproj_sti(strip - 2, sti)
            for l in range(nl):
                emit_attention_head(strip, l)
                if l == 1:
                    emit_ag(strip, 0)
            if strip == sq - 1:
                for sti in range(4):
                    emit_outproj_sti(sq - 2, sti)
            emit_ag(strip, 1)
        for sti in range(4):
            emit_outproj_sti(sq - 1, sti)


def make_inputs(x, W_qkv, b_qkv, W_out, b_out, s=S, h=H, nh=NH):
    """Host-side sharding: per-core input dicts (layout prep only)."""
    nl = nh // GROUPS
    dg = nl * D
    hc = h // 128
    sq = s // 512
    bf16 = ml_dtypes.bfloat16
    x = np.asarray(x, dtype=np.float32)
    W_qkv = np.asarray(W_qkv, dtype=np.float32)
    b_qkv = np.asarray(b_qkv, dtype=np.float32)
    W_out = np.asarray(W_out, dtype=np.float32)
    b_out = np.asarray(b_out, dtype=np.float32)

    # causal staircase master mask: mask[i, u] = 1 iff u >= i + 384
    uu = np.arange(896)[None, :]
    ii = np.arange(128)[:, None]
    mask = (uu >= ii + 384).astype(bf16)
    ones = np.ones((128, 128), dtype=np.float32)

    WoT = W_out.T  # [h (d-in), h (n-out)]
    in_maps = []
    for core in range(NCORES):
        b, g = divmod(core, GROUPS)
        xT = x[b].T                                   # [h, s]
        # x strips: [sq, 128, hc, 512]
        xs = np.ascontiguousarray(
            xT.reshape(hc, 128, sq, 512).transpose(2, 1, 0, 3).astype(bf16))

        def arr_w(wslice, dt):
            # [dg, h] -> transposed chunks [128, hc, dg]
            return np.ascontiguousarray(
                wslice.T.reshape(hc, 128, dg).transpose(1, 0, 2).astype(dt))

        wq = arr_w(W_qkv[dg * g:dg * (g + 1), :], bf16)
        wk = arr_w(W_qkv[h + dg * g:h + dg * (g + 1), :], bf16)
        wv = arr_w(W_qkv[2 * h + dg * g:2 * h + dg * (g + 1), :], bf16)
        bq = np.ascontiguousarray(
            b_qkv[dg * g:dg * (g + 1)].reshape(nl, 128).T)      # [128, nl]
        bk = np.ascontiguousarray(
            b_qkv[h + dg * g:h + dg * (g + 1)].reshape(nl, 128).T)
        bv = np.tile(b_qkv[2 * h + dg * g:2 * h + dg * (g + 1)][None, :], (128, 1))
        bo = np.tile(b_out[dg * g:dg * (g + 1)][None, :], (128, 1))
        # W_out^T rows permuted to the per-pair AG d-order:
        # chunk c = 8*pair + 2*r + lp  <->  global head 4r + 2*pair + lp
        blocks = []
        for pair in range(2):
            for r in range(GROUPS):
                for lp in range(2):
                    hh = nl * r + 2 * pair + lp
                    blocks.append(WoT[D * hh:D * (hh + 1), dg * g:dg * (g + 1)])
        wo = np.ascontiguousarray(
            np.concatenate(blocks, axis=0)
            .reshape(hc, 128, dg).transpose(1, 0, 2).astype(np.float16))
        in_maps.append({
            "x": xs, "wq": wq, "wk": wk, "wv": wv, "wo": wo,
            "bq": bq, "bk": bk,
            "bv": np.ascontiguousarray(bv.astype(np.float32)),
            "bo": np.ascontiguousarray(bo.astype(np.float32)),
            "mask": mask, "ones": ones,
        })
    return in_maps


_NC_CACHE = {}


def _get_nc(key=(S, H, NH)):
    if key not in _NC_CACHE:
        _NC_CACHE[key] = build_nc(*key)
    return _NC_CACHE[key]


def kernel(x, W_qkv, b_qkv, W_out, b_out):
    global LAST_EXEC_NS, LAST_RESULTS
    nc = _get_nc()
    in_maps = make_inputs(x, W_qkv, b_qkv, W_out, b_out)
    res = run_bass_kernel_spmd(
        nc, in_maps, core_ids=list(range(NCORES)), trace=TRACE)
    LAST_EXEC_NS = res.exec_time_ns
    LAST_RESULTS = res
    out = np.empty((B, S, H), dtype=np.float32)
    for core in range(NCORES):
        b, g = divmod(core, GROUPS)
        out[b, :, DG * g:DG * (g + 1)] = res.results[core]["out"]
    return out


# revision 39
# speedup vs baseline: 1.1128x; 1.0333x over previous
"""Trainium2 Bass kernel: fused multi-head causal self-attention block.

Computes, for x:(B,S,H), W_qkv:(3H,H), b_qkv:(3H,), W_out:(H,H), b_out:(H,):
    qkv = x @ W_qkv.T + b_qkv ; split into q,k,v heads (NH heads, D=H/NH)
    out = softmax(causal(q k^T / sqrt(D))) v   ; merge heads
    return out @ W_out.T + b_out

Sharding over 8 NeuronCores: DP(2 batches) x TP(4 head-groups).
Core c handles batch b=c//4, head group g=c%4 (heads 4g..4g+3).

v2 design (single fused strip pipeline):
  - All matmul tensors bf16 (same PE rate as fp32r, half the DMA/SBUF,
    FWL-accelerated weight loads). PSUM accumulation stays fp32.
  - x is loaded once, host-prearranged per 512-column strip; Q^T/K^T and
    V projections run as PSUM-chained accumulations (2 banks total).
  - Attention runs strip-outer / head-inner; each strip's normalized A^T
    ([4*128, 512] f16) is AllGather'd across the 4-core batch group as
    soon as the strip finishes, so the output projection for strip s-1
    overlaps attention of strip s (interleaved at head granularity).
  - softmax denominator accumulated on the Vector engine (DVE) and
    contracted with a single ones-vector matmul per (head, strip);
    reciprocal is taken on the [1,512] row then broadcast via a 1-row
    matmul (cheap) instead of a [128,512] reciprocal (very slow).
  - Diagonal score tiles restrict the q-column range to the unmasked
    staircase, skipping fully-masked columns in scores/exp/AV.
Each core computes a disjoint 512-column slice of the output, so the
host does a pure concatenation.
"""

import math

import numpy as np
import ml_dtypes

import concourse.bass as bass
import concourse.mybir as mybir
import concourse.tile as tile
from concourse import bacc, bass_isa
from concourse.bass_utils import run_bass_kernel_spmd

FP = mybir.dt.float32
FR = mybir.dt.float32r
BF = mybir.dt.bfloat16
F16 = mybir.dt.float16

# Full-size problem constants.
B, S, H, NH = 2, 2048, 2048, 16
D = 128
NCORES = 8
GROUPS = 4                  # head-groups per batch (TP degree)
NL = NH // GROUPS           # local heads per core
DG = NL * D                 # per-core slice of the head dim
REPLICA_GROUPS = [[0, 1, 2, 3], [4, 5, 6, 7]]

TRACE = False               # set by test harness to capture NTFF profile
LAST_EXEC_NS = None
LAST_RESULTS = None


def build_nc(s=S, h=H, nh=NH, reps=1, ag=True):
    """Build the SPMD Bass program (identical on all 8 cores)."""
    nc = bacc.Bacc(
        "TRN2",
        target_bir_lowering=False,
        debug=False,
        enable_asserts=False,
        num_devices=NCORES,
    )

    nl = nh // GROUPS
    dg = nl * D
    hc = h // 128               # contraction chunks
    sq = s // 512               # 512-wide strips

    # ---- I/O (all host-prearranged for contiguous DMA) ----------------
    # x strips: [strip, 128, hc, 512] bf16 : x[st, p, c, t] = xT[128c+p, 512st+t]
    x_d = nc.dram_tensor("x", [sq, 128, hc, 512], BF, kind="ExternalInput")
    # weights: [128, hc, dg] : w[p, c, d] = W^T[128c+p, d]
    wq_d = nc.dram_tensor("wq", [128, hc, dg], BF, kind="ExternalInput")
    wk_d = nc.dram_tensor("wk", [128, hc, dg], BF, kind="ExternalInput")
    wv_d = nc.dram_tensor("wv", [128, hc, dg], BF, kind="ExternalInput")
    # out-proj weights, rows permuted to AG order: chunk c=(4r+l) <-> head 4r+l
    wo_d = nc.dram_tensor("wo", [128, hc, dg], F16, kind="ExternalInput")
    bq_d = nc.dram_tensor("bq", [128, nl], FP, kind="ExternalInput")
    bk_d = nc.dram_tensor("bk", [128, nl], FP, kind="ExternalInput")
    bv_d = nc.dram_tensor("bv", [128, dg], FP, kind="ExternalInput")
    bo_d = nc.dram_tensor("bo", [128, dg], FP, kind="ExternalInput")
    mask_d = nc.dram_tensor("mask", [128, 896], BF, kind="ExternalInput")
    ones_d = nc.dram_tensor("ones", [128, 128], FR, kind="ExternalInput")
    out_d = nc.dram_tensor("out", [s, dg], FP, kind="ExternalOutput")

    with tile.TileContext(nc) as tc:
        with tc.tile_pool(name="const", bufs=1) as constp:
            mask_sb = constp.tile([128, 896], BF)
            nc.sync.dma_start(mask_sb[:], mask_d[:])
            ones_sb = constp.tile([128, 128], FR)
            nc.sync.dma_start(ones_sb[:], ones_d[:])
            bq_sb = constp.tile([128, nl], FP)
            nc.sync.dma_start(bq_sb[:], bq_d[:])
            bk_sb = constp.tile([128, nl], FP)
            nc.sync.dma_start(bk_sb[:], bk_d[:])
            bv_sb = constp.tile([128, dg], FP)
            nc.sync.dma_start(bv_sb[:], bv_d[:])
            bo_sb = constp.tile([128, dg], FP)
            nc.sync.dma_start(bo_sb[:], bo_d[:])

            for _rep in range(reps):
                _emit_body(nc, tc, s, h, nh,
                           x_d, wq_d, wk_d, wv_d, wo_d, out_d,
                           bq_sb, bk_sb, bv_sb, bo_sb,
                           mask_sb, ones_sb, ag)

    nc.compile()
    return nc


def _emit_body(nc, tc, s, h, nh,
               x_d, wq_d, wk_d, wv_d, wo_d, out_d,
               bq_sb, bk_sb, bv_sb, bo_sb, mask_sb, ones_sb, ag=True):
    nl = nh // GROUPS
    dg = nl * D
    hc = h // 128
    sq = s // 512
    st_n = s // 128
    scale = 1.0 / math.sqrt(D)
    ones_col = ones_sb[:, 0:1]   # [128,1] FR: denominator partition-contract
    ones_row = ones_sb[0:1, :]   # [1,128] FR: partition broadcast

    with tc.tile_pool(name="wts", bufs=1) as wtp, \
         tc.tile_pool(name="xp", bufs=2) as xp, \
         tc.tile_pool(name="qkv", bufs=1) as qkvp, \
         tc.tile_pool(name="atrp", bufs=2) as atrp, \
         tc.tile_pool(name="etp", bufs=3) as etp, \
         tc.tile_pool(name="anp", bufs=2) as anp, \
         tc.tile_pool(name="dnp", bufs=2) as dnp, \
         tc.tile_pool(name="obp", bufs=1) as obp, \
         tc.tile_pool(name="dramp", bufs=1, space="DRAM") as dramp, \
         tc.tile_pool(name="psCH", bufs=2, space="PSUM") as psCH, \
         tc.tile_pool(name="psS", bufs=2, space="PSUM") as psS, \
         tc.tile_pool(name="psAV", bufs=2, space="PSUM") as psAV, \
         tc.tile_pool(name="psDR", bufs=1, space="PSUM") as psDR, \
         tc.tile_pool(name="psO", bufs=1, space="PSUM") as psO:

        # ---- persistent SBUF tensors -----------------------------------
        qT = [qkvp.tile([128, s], BF, tag=f"qT{t}", name=f"qT{t}") for t in range(nl)]
        kT = [qkvp.tile([128, s], BF, tag=f"kT{t}", name=f"kT{t}") for t in range(nl)]
        vv = [qkvp.tile([128, dg], BF, tag=f"v{t}", name=f"v{t}") for t in range(st_n)]

        x_sbs = {}

        def emit_x_load(strip):
            x_sb = xp.tile([128, hc, 512], BF, tag="xs", name="xs")
            # per-chunk sub-DMAs so the first chains can start early
            for c in range(hc):
                nc.sync.dma_start(x_sb[:, c, :], x_d[strip, :, c, :])
            x_sbs[strip] = x_sb

        # interleave strip-0 x with the Q/K weights so the first chain's
        # per-chunk matmuls can start as soon as their operands land
        wq_sb = wtp.tile([128, hc, dg], BF, tag="wq", name="wq_sb")
        wk_sb = wtp.tile([128, hc, dg], BF, tag="wk", name="wk_sb")
        wv_sb = wtp.tile([128, hc, dg], BF, tag="wv", name="wv_sb")
        wo_sb = wtp.tile([128, hc, dg], F16, tag="wo", name="wo_sb")
        x0_sb = xp.tile([128, hc, 512], BF, tag="xs", name="xs0")
        for b in range(4):
            cb = slice(4 * b, 4 * b + 4)
            nc.sync.dma_start(wq_sb[:, cb, :], wq_d[:, cb, :])
            nc.scalar.dma_start(wk_sb[:, cb, :], wk_d[:, cb, :])
            for c in range(4 * b, 4 * b + 4):
                nc.sync.dma_start(x0_sb[:, c, :], x_d[0, :, c, :])
        x_sbs[0] = x0_sb
        nc.scalar.dma_start(wv_sb[:], wv_d[:])
        nc.scalar.dma_start(wo_sb[:], wo_d[:])

        # AG buffers (DRAM), split by head pair: pair p holds heads 2p,2p+1
        agin = [[dramp.tile([2 * 128, 512], F16, tag=f"agin{st}_{p}",
                            name=f"agin{st}_{p}") for p in range(2)]
                for st in range(sq)]
        agout = [[dramp.tile([4 * 256, 512], F16, tag=f"agout{st}_{p}",
                             name=f"agout{st}_{p}") for p in range(2)]
                 for st in range(sq)]

        def emit_qkv(strip):
            """Q^T,K^T,V projections for one 512-col strip of the sequence."""
            cs = slice(512 * strip, 512 * strip + 512)
            x_sb = x_sbs.pop(strip)
            # Q/K chains: one [128,512] psum accumulated over all hc chunks
            for gi in range(2 * nl):
                is_q = gi % 2 == 0          # interleave Q,K per head
                t = gi // 2
                w_sb = wq_sb if is_q else wk_sb
                ps = psCH.tile([128, 512], FP, tag="chain", name="ps_qk")
                for c in range(hc):
                    nc.tensor.matmul(
                        ps[:],
                        w_sb[:, c, 128 * t:128 * t + 128],
                        x_sb[:, c, :],
                        start=(c == 0), stop=(c == hc - 1),
                    )
                if is_q:
                    nc.scalar.activation(
                        qT[t][:, cs], ps[:],
                        mybir.ActivationFunctionType.Identity,
                        bias=bq_sb[:, t:t + 1],
                    )
                else:
                    nc.vector.tensor_scalar_add(kT[t][:, cs], ps[:], bk_sb[:, t:t + 1])
            # V chains: natural [s,d] layout, one per 128-row s-tile
            for sti in range(4):
                st_idx = 4 * strip + sti
                ps = psCH.tile([128, dg], FP, tag="chain", name="ps_v")
                for c in range(hc):
                    nc.tensor.matmul(
                        ps[:],
                        x_sb[:, c, 128 * sti:128 * sti + 128],
                        wv_sb[:, c, :],
                        start=(c == 0), stop=(c == hc - 1),
                    )
                nc.vector.tensor_add(vv[st_idx][:], ps[:], bv_sb[:])

        def emit_attention_head(strip, l):
            """Causal attention for head l restricted to q-strip `strip`."""
            nk = 4 * strip + 4
            ps_av = psAV.tile([128, 512], FP, tag="ps_av", name="ps_av")
            dn_acc = dnp.tile([128, 512], FR, tag="dn_acc", name="dn_acc")
            ets = []
            # software-pipelined: scores(kt) ... AV(kt-1) so exp can run ahead
            for kt in range(nk + 1):
                if kt < nk:
                    c = kt - 4 * strip          # >=0 on diagonal tiles
                    qc = slice(128 * c, 512) if c >= 0 else slice(0, 512)
                    qg = slice(512 * strip + qc.start, 512 * strip + 512)
                    ps_s = psS.tile([128, 512], FP, tag="ps_s", name="ps_s")
                    nc.tensor.matmul(
                        ps_s[:, qc],
                        kT[l][:, 128 * kt:128 * kt + 128],
                        qT[l][:, qg],
                        start=True, stop=True,
                    )
                    et = etp.tile([128, 512], BF, tag="et", name="et")
                    nc.scalar.activation(
                        et[:, qc], ps_s[:, qc],
                        mybir.ActivationFunctionType.Exp,
                        scale=scale,
                    )
                    if c >= 0:
                        nc.vector.tensor_mul(
                            et[:, qc], et[:, qc], mask_sb[:, 384:896 - 128 * c])
                    # denominator accumulate on DVE
                    if kt == 0:
                        nc.vector.tensor_copy(dn_acc[:, qc], et[:, qc])
                    else:
                        nc.vector.tensor_add(dn_acc[:, qc], dn_acc[:, qc], et[:, qc])
                    ets.append((et, qc))
                if kt >= 1:
                    et, qc = ets[kt - 1]
                    nc.tensor.matmul(
                        ps_av[:, qc],
                        vv[kt - 1][:, 128 * l:128 * l + 128],
                        et[:, qc],
                        start=(kt - 1 == 0), stop=(kt - 1 == nk - 1),
                    )
            # denominator: ones-contract over partitions (PE), fast approx
            # reciprocal on the [1,512] row (DVE), ones-broadcast back to
            # [128,512] (PE), then normalize. ACT stages the PSUM->SBUF hops.
            ps_dn = psDR.tile([1, 512], FP, tag="dnrb", name="ps_dn")
            nc.tensor.matmul(ps_dn[:], ones_col, dn_acc[:], start=True, stop=True)
            dn_f32 = dnp.tile([1, 512], FP, tag="dn_f32", name="dn_f32")
            nc.scalar.copy(dn_f32[:], ps_dn[:])
            rec_f32 = dnp.tile([1, 512], FP, tag="rec_f32", name="rec_f32")
            nc.vector.reciprocal_approx_fast(rec_f32[:], dn_f32[:])
            rec_fr = dnp.tile([1, 512], FR, tag="rec_fr", name="rec_fr")
            nc.vector.tensor_copy(rec_fr[:], rec_f32[:])
            ps_rb = psDR.tile([128, 512], FP, tag="dnrb", name="ps_rb")
            nc.tensor.matmul(ps_rb[:], ones_row, rec_fr[:], start=True, stop=True)
            rb_sb = dnp.tile([128, 512], FP, tag="rb_sb", name="rb_sb", bufs=1)
            nc.scalar.copy(rb_sb[:], ps_rb[:])
            an = anp.tile([128, 512], F16, tag="an", name="an")
            nc.vector.tensor_mul(an[:], ps_av[:], rb_sb[:])
            # ship this head's strip slice to the AG input buffer
            nc.sync.dma_start(
                agin[strip][l // 2][128 * (l % 2):128 * (l % 2) + 128, :], an[:])

        at_p = {}

        def emit_ag_trigger(strip, pair):
            if ag:
                nc.gpsimd.collective_compute(
                    "AllGather",
                    mybir.AluOpType.bypass,
                    replica_groups=REPLICA_GROUPS,
                    ins=[agin[strip][pair].opt()],
                    outs=[agout[strip][pair].opt()],
                )
            else:
                nc.gpsimd.dma_start(agout[strip][pair][0:256, :],
                                    agin[strip][pair][:])

        def emit_atr_loads(strip, pair):
            """SBUF loads of the 4 rank blocks of one pair's AG output.

            On the gpsimd queue: these are the only consumers of collective
            completion, so their waits block nothing else. Anything emitted
            after them on any queue inherits a wait for them, so outproj for
            an older strip must be emitted before these."""
            for r in range(GROUPS):
                t = atrp.tile([128, 2, 512], F16, tag=f"atr{pair}{r}",
                              name=f"atr{pair}{r}")
                nc.gpsimd.dma_start(
                    t[:],
                    agout[strip][pair][256 * r:256 * r + 256, :]
                    .rearrange("(l p) t -> p l t", p=128))
                at_p[(strip, pair, r)] = t

        def emit_ag(strip, pair):
            emit_ag_trigger(strip, pair)
            emit_atr_loads(strip, pair)

        def emit_outproj_sti(strip, sti):
            """One 128-row s-tile of the output projection for `strip`."""
            rs = slice(512 * strip + 128 * sti, 512 * strip + 128 * sti + 128)
            ssl = slice(128 * sti, 128 * sti + 128)
            ps_o = psO.tile([128, dg], FP, tag="ps_o", name="ps_o")
            for c in range(hc):
                pair, rl = divmod(c, 2 * GROUPS)
                r, lp = divmod(rl, 2)
                lhsT = at_p[(strip, pair, r)][:, lp, ssl]
                nc.tensor.matmul(
                    ps_o[:], lhsT, wo_sb[:, c, :],
                    start=(c == 0), stop=(c == hc - 1),
                )
            ob = obp.tile([128, dg], FP, tag="ob", name="ob")
            nc.vector.tensor_add(ob[:], ps_o[:], bo_sb[:])
            nc.sync.dma_start(out_d[rs, :], ob[:])

        # ---- main fused pipeline ---------------------------------------
        # Each head pair's AllGather fires as soon as both heads finish, and
        # outproj for strip s runs right after strip s+1's attention — after
        # its atr loads are emitted (and long since landed), before strip
        # s+1's own atr loads, so queue-threshold waits stay tight.
        for strip in range(sq):
            last = strip == sq - 1
            if not last:
                emit_x_load(strip + 1)
            emit_qkv(strip)
            if strip >= 2:
                for sti in range(4):
                    emit_outproj_sti(strip - 2, sti)
            for l in range(nl):
                emit_attention_head(strip, l)
                if l == 1:
                    # last strip: defer the atr loads so the strip-2 outproj
                    # emitted below doesn't inherit a wait on this AG
                    emit_ag_trigger(strip, 0)
                    if not last:
                        emit_atr_loads(strip, 0)
            if last:
                for sti in range(4):
                    emit_outproj_sti(sq - 2, sti)
                emit_atr_loads(strip, 0)
            emit_ag(strip, 1)
        for sti in range(4):
            emit_outproj_sti(sq - 1, sti)


def make_inputs(x, W_qkv, b_qkv, W_out, b_out, s=S, h=H, nh=NH):
    """Host-side sharding: per-core input dicts (layout prep only)."""
    nl = nh // GROUPS
    dg = nl * D
    hc = h // 128
    sq = s // 512
    bf16 = ml_dtypes.bfloat16
    x = np.asarray(x, dtype=np.float32)
    W_qkv = np.asarray(W_qkv, dtype=np.float32)
    b_qkv = np.asarray(b_qkv, dtype=np.float32)
    W_out = np.asarray(W_out, dtype=np.float32)
    b_out = np.asarray(b_out, dtype=np.float32)

    # causal staircase master mask: mask[i, u] = 1 iff u >= i + 384
    uu = np.arange(896)[None, :]
    ii = np.arange(128)[:, None]
    mask = (uu >= ii + 384).astype(bf16)
    ones = np.ones((128, 128), dtype=np.float32)

    WoT = W_out.T  # [h (d-in), h (n-out)]
    in_maps = []
    for core in range(NCORES):
        b, g = divmod(core, GROUPS)
        xT = x[b].T                                   # [h, s]
        # x strips: [sq, 128, hc, 512]
        xs = np.ascontiguousarray(
            xT.reshape(hc, 128, sq, 512).transpose(2, 1, 0, 3).astype(bf16))

        def arr_w(wslice, dt):
            # [dg, h] -> transposed chunks [128, hc, dg]
            return np.ascontiguousarray(
                wslice.T.reshape(hc, 128, dg).transpose(1, 0, 2).astype(dt))

        wq = arr_w(W_qkv[dg * g:dg * (g + 1), :], bf16)
        wk = arr_w(W_qkv[h + dg * g:h + dg * (g + 1), :], bf16)
        wv = arr_w(W_qkv[2 * h + dg * g:2 * h + dg * (g + 1), :], bf16)
        bq = np.ascontiguousarray(
            b_qkv[dg * g:dg * (g + 1)].reshape(nl, 128).T)      # [128, nl]
        bk = np.ascontiguousarray(
            b_qkv[h + dg * g:h + dg * (g + 1)].reshape(nl, 128).T)
        bv = np.tile(b_qkv[2 * h + dg * g:2 * h + dg * (g + 1)][None, :], (128, 1))
        bo = np.tile(b_out[dg * g:dg * (g + 1)][None, :], (128, 1))
        # W_out^T rows permuted to the per-pair AG d-order:
        # chunk c = 8*pair + 2*r + lp  <->  global head 4r + 2*pair + lp
        blocks = []
        for pair in range(2):
            for r in range(GROUPS):
                for lp in range(2):
                    hh = nl * r + 2 * pair + lp
                    blocks.append(WoT[D * hh:D * (hh + 1), dg * g:dg * (g + 1)])
        wo = np.ascontiguousarray(
            np.concatenate(blocks, axis=0)
            .reshape(hc, 128, dg).transpose(1, 0, 2).astype(np.float16))
        in_maps.append({
            "x": xs, "wq": wq, "wk": wk, "wv": wv, "wo": wo,
            "bq": bq, "bk": bk,
            "bv": np.ascontiguousarray(bv.astype(np.float32)),
            "bo": np.ascontiguousarray(bo.astype(np.float32)),
            "mask": mask, "ones": ones,
        })
    return in_maps


_NC_CACHE = {}


def _get_nc(key=(S, H, NH)):
    if key not in _NC_CACHE:
        _NC_CACHE[key] = build_nc(*key)
    return _NC_CACHE[key]


def kernel(x, W_qkv, b_qkv, W_out, b_out):
    global LAST_EXEC_NS, LAST_RESULTS
    nc = _get_nc()
    in_maps = make_inputs(x, W_qkv, b_qkv, W_out, b_out)
    res = run_bass_kernel_spmd(
        nc, in_maps, core_ids=list(range(NCORES)), trace=TRACE)
    LAST_EXEC_NS = res.exec_time_ns
    LAST_RESULTS = res
    out = np.empty((B, S, H), dtype=np.float32)
    for core in range(NCORES):
        b, g = divmod(core, GROUPS)
        out[b, :, DG * g:DG * (g + 1)] = res.results[core]["out"]
    return out
